# revision 20
# baseline (speedup 1.0000x reference)
"""Trainium2 Bass kernel for NonLocalCA (embedded-gaussian non-local block on
2x2 quadrants with shared BatchNorm over the batch axis).

Problem shapes (hardcoded): x [B=2, C=64, H=128, W=128], Ci=32.
Each of the 4 quadrants is an independent 4096-token attention over both batch
elements; BatchNorm couples the two batch elements of a quadrant.

Sharding: 8 cores = 4 quadrants x 2 batch elements. Core k handles quadrant
k//2, batch k%2 and computes the full [4096, 4096] attention for its block.
The only cross-core communication is the BatchNorm (sum, sumsq) allreduce
between the two cores of a quadrant (replica groups [[0,1],[2,3],[4,5],[6,7]]).

Math per core (xf = quadrant tokens [C=64, N=4096], aug = ones row appended):
  th_rep [128, N] = TH_REP.T @ xf_aug   (4 stacked copies of theta proj + bias)
  ph_rep [128, N] = PH_REP.T @ xf_aug   (4 stacked copies of phi proj + bias)
  gxT    [N, 33]  = xf_aug.T @ G_AUG    (g proj + bias, 33rd col = ones)
  per 512-wide query block n, per 128-token key block m:
    fT[m, n-block] = ph[:, m-block].T @ th[:, n-block]      (PE, K=32)
    aT = exp(fT)                                            (ACT, PSUM->SBUF)
    yT_aug[33, n-block] += gxT[m-block].T @ aT              (PE, K=128)
  row 32 of yT_aug is the softmax denominator (ones-column trick); normalize
  after the W projection:  wy = (WT.T @ yT[0:32]) * (1/denom broadcast).
  BatchNorm stats of wy are allreduced with the sibling core, then
  out = wy*scale + (beta - mean*scale) + xf  (w_b cancels inside BN).
"""

import numpy as np

import concourse.bass as bass
import concourse.mybir as mybir
import concourse.tile as tile
from concourse import bacc
from concourse.bass_utils import run_bass_kernel_spmd

F32 = mybir.dt.float32
LOWP = mybir.dt.float16
AF = mybir.ActivationFunctionType
ALU = mybir.AluOpType

B, C, H, W = 2, 64, 128, 128
CI = 32
HQ = H // 2  # 64
N_FULL = HQ * HQ  # 4096 tokens per quadrant
NB = 512  # query-block width (one PSUM bank of fp32)
MBLK = 128  # key-block height (partition dim)
GRP = 3  # key blocks per exp chunk (3 PSUM banks per fT tile)
BN_EPS = 1e-5


def build_nc(n_tokens=N_FULL, n_cores=8, with_collective=True, pack_mm1=True,
             cc_kind="ag_pair"):
    """Build the SPMD Bass module. n_tokens < 4096 gives a small variant for
    simulation. cc_kind: 'ar_pair' (pairwise AllReduce) or 'ag_pair'
    (pairwise AllGather + local add). Returns the compiled Bacc object."""
    NT = n_tokens
    n_nb = NT // NB  # query blocks
    n_mb = NT // MBLK  # key blocks
    bn_count = (2 if with_collective else 1) * NT

    nc = bacc.Bacc(
        "TRN2", target_bir_lowering=False, debug=False, num_devices=n_cores
    )

    xq_d = nc.dram_tensor("xq", [C + 1, NT], F32, kind="ExternalInput")
    xqlp_d = nc.dram_tensor("xqlp", [C + 1, NT], LOWP, kind="ExternalInput")
    threp_d = nc.dram_tensor("threp", [C + 1, 128], LOWP, kind="ExternalInput")
    phrep_d = nc.dram_tensor("phrep", [C + 1, 128], LOWP, kind="ExternalInput")
    gaug_d = nc.dram_tensor("gaug", [C + 1, CI + 1], LOWP, kind="ExternalInput")
    wt_d = nc.dram_tensor("wt", [CI, C], LOWP, kind="ExternalInput")
    bnp_d = nc.dram_tensor("bnp", [C, 2], F32, kind="ExternalInput")
    out_d = nc.dram_tensor("out", [C, NT], F32, kind="ExternalOutput")
    if with_collective:
        out_rows = 2 * C if cc_kind == "ag_pair" else C
        ccw_in = nc.dram_tensor("ccw_in", [C, 2], F32)
        ccw_out = nc.dram_tensor("ccw_out", [out_rows, 2], F32)
        cca_in = nc.dram_tensor("cca_in", [C, 2], F32)
        cca_out = nc.dram_tensor("cca_out", [out_rows, 2], F32)
        ccb_in = nc.dram_tensor("ccb_in", [C, 2], F32)
        ccb_out = nc.dram_tensor("ccb_out", [out_rows, 2], F32)
        groups = [[2 * q, 2 * q + 1] for q in range(n_cores // 2)]

        def emit_cc(cci, cco):
            if cc_kind == "ag_pair":
                nc.gpsimd.collective_compute(
                    "AllGather",
                    ALU.bypass,
                    replica_groups=groups,
                    ins=[cci[:, :]],
                    outs=[cco[:, :]],
                )
            else:
                nc.gpsimd.collective_compute(
                    "AllReduce",
                    ALU.add,
                    replica_groups=groups,
                    ins=[cci[:, :]],
                    outs=[cco[:, :]],
                )

    with tile.TileContext(nc) as tc:
        with (
            tc.tile_pool(name="consts", bufs=1) as consts,
            tc.tile_pool(name="small", bufs=4) as small,
            tc.tile_pool(name="atp", bufs=4) as atp,
            tc.tile_pool(name="outp", bufs=3) as outp,
            tc.tile_pool(name="pf", bufs=2, space="PSUM") as pf,
            tc.tile_pool(name="py", bufs=2, space="PSUM") as py,
        ):
            # ---- load weights ----
            threp_w = consts.tile([C + 1, 128], LOWP, tag="threp_w")
            nc.gpsimd.dma_start(out=threp_w, in_=threp_d[:, :])
            phrep_w = consts.tile([C + 1, 128], LOWP, tag="phrep_w")
            nc.gpsimd.dma_start(out=phrep_w, in_=phrep_d[:, :])
            gaug = consts.tile([C + 1, CI + 1], LOWP, tag="gaug")
            nc.gpsimd.dma_start(out=gaug, in_=gaug_d[:, :])
            wt = consts.tile([CI, C], LOWP, tag="wt")
            nc.gpsimd.dma_start(out=wt, in_=wt_d[:, :])
            bnp = consts.tile([C, 2], F32, tag="bnp")
            nc.gpsimd.dma_start(out=bnp, in_=bnp_d[:, :])

            # warmup collective: absorbs cross-core NEFF-launch skew and
            # warms the CC rings so the real stats collectives at the tail
            # see minimal latency. Runs concurrently with the input DMAs /
            # prologue; nothing waits on its result.
            if with_collective:
                nc.sync.dma_start(out=ccw_in[:, :], in_=bnp)
                emit_cc(ccw_in, ccw_out)

            # ---- input load + projections, pipelined in column chunks ----
            # th_rep / ph_rep: [128, NT] bf16, rows 32i+j = proj row j (4 copies)
            xf = consts.tile([C + 1, NT], F32, tag="xf")
            xflp = consts.tile([C + 1, NT], LOWP, tag="xflp")
            th_rep = consts.tile([128, NT], LOWP, tag="th_rep")
            ph_rep = consts.tile([128, NT], LOWP, tag="ph_rep")
            gxT = consts.tile([128, (CI + 1) * n_mb], LOWP, tag="gxT")
            mb_per_nb = NB // MBLK  # 4 key blocks per column chunk

            def emit_prologue_dma(c0):
                csz = min(GRP, n_nb - c0)
                cs = slice(c0 * NB, (c0 + csz) * NB)
                if c0 == 0:  # split the first chunk so projections start sooner
                    for j in range(csz):
                        js = slice(j * NB, (j + 1) * NB)
                        nc.sync.dma_start(out=xflp[:, js], in_=xqlp_d[:, js])
                    nc.gpsimd.dma_start(out=xf[:, cs], in_=xq_d[:, cs])
                else:
                    nc.sync.dma_start(out=xflp[:, cs], in_=xqlp_d[:, cs])
                    nc.gpsimd.dma_start(out=xf[:, cs], in_=xq_d[:, cs])

            def emit_prologue_chunk(c0, with_dma=True):
                csz = min(GRP, n_nb - c0)
                cs = slice(c0 * NB, (c0 + csz) * NB)
                if with_dma:
                    emit_prologue_dma(c0)
                for dst, w in ((th_rep, threp_w), (ph_rep, phrep_w)):
                    ps = pf.tile([128, GRP * NB], F32, tag="f", name="ps_proj")
                    for j in range(csz):
                        nc.tensor.matmul(
                            ps[:, j * NB : (j + 1) * NB],
                            w,
                            xflp[:, (c0 + j) * NB : (c0 + j + 1) * NB],
                            start=True,
                            stop=True,
                        )
                    if c0 == 0:  # ACT is idle before the first exp
                        nc.scalar.copy(dst[:, cs], ps[:, : csz * NB])
                    else:  # keep ACT free for exp once the main loop runs
                        nc.vector.tensor_copy(dst[:, cs], ps[:, : csz * NB])
                # gxT blocks covered by this column chunk
                m0 = c0 * mb_per_nb
                bsz = csz * mb_per_nb
                ps = pf.tile([128, GRP * NB], F32, tag="f", name="ps_gxt")
                for j in range(bsz):
                    nc.tensor.matmul(
                        ps[:, j * (CI + 1) : (j + 1) * (CI + 1)],
                        xflp[:, (m0 + j) * MBLK : (m0 + j + 1) * MBLK],
                        gaug,
                        start=True,
                        stop=True,
                    )
                nc.vector.tensor_copy(
                    gxT[:, m0 * (CI + 1) : (m0 + bsz) * (CI + 1)],
                    ps[:, : bsz * (CI + 1)],
                )

            emit_prologue_chunk(0)
            # input DMAs for the deferred chunks go up front (queue-bandwidth
            # bound, no reason to wait); only the projection matmuls are
            # deferred into block 0's stream
            for dc in range(GRP, n_nb, GRP):
                emit_prologue_dma(dc)
            deferred_chunks = list(range(GRP, n_nb, GRP))

            # ---- main attention loop ----
            wy_full = consts.tile([C, NT], F32, tag="wy_full")
            bnst = consts.tile([C, n_nb, 6], F32, tag="bnst")

            def emit_wy_tail(nb, y_sb, denb):  # y_sb: [CI, NB] fp16
                """W projection + normalize + BN partial stats for block nb.
                Deferred into the next block's attention stream so the PE
                never stalls waiting on the DVE y-chain."""
                nsl = slice(nb * NB, (nb + 1) * NB)
                wyps = py.tile([C, NB], F32, tag="y", name="wyps")
                nc.tensor.matmul(wyps, wt, y_sb, start=True, stop=True)
                nc.vector.tensor_mul(wy_full[:, nsl], wyps, denb)
                nc.vector.bn_stats(
                    out=bnst[:, nb, :], in_=wy_full[:, nsl]
                )

            def emit_ychain(nb, yps_a, yps_b):
                """stripA+stripB, fp16 copy for the W matmul, and the
                reciprocal-of-denominator chain (DVE + GpSimd only)."""
                y_sb = small.tile([CI + 1, NB], F32, tag="y_sb")
                nc.vector.tensor_copy(y_sb, yps_a[0 : CI + 1, :])
                nc.vector.tensor_add(y_sb, y_sb, yps_b[64 : 64 + CI + 1, :])
                y16 = small.tile([CI, NB], LOWP, tag="y16")
                nc.vector.tensor_copy(y16, y_sb[0:CI, :])
                # reciprocal of the denominator row: reshape [1,NB] ->
                # [128,NB/128] via two small SBUF->SBUF DMAs so all DVE
                # lanes participate
                rr_in = small.tile([128, NB // 128], F32, tag="rr_in")
                nc.sync.dma_start(out=rr_in, in_=y_sb[CI : CI + 1, :])
                rr4 = small.tile([128, NB // 128], F32, tag="rr4")
                nc.vector.reciprocal(rr4, rr_in)
                recip = small.tile([1, NB], F32, tag="recip")
                nc.sync.dma_start(out=recip, in_=rr4)
                denb = small.tile([C, NB], F32, tag="denb")
                nc.gpsimd.partition_broadcast(denb, recip)
                return (nb, y16, denb)

            # one-group-deep software pipeline across the whole stream: the PE
            # order is ... mm1(k), [mm2(k-1)], mm1(k+1), ... and each block's
            # last mm2 group + y-chain + W-projection slide into the next
            # block's stream so neither PE nor ACT ever waits at a boundary.
            pqueue = []  # (at, g0, gsz, yps_a, yps_b) awaiting mm2, depth 2
            ychain = None  # (yps pair, nb) awaiting stripA+stripB+recip
            pending_wy = None  # (nb, y16, denb) awaiting W projection + stats

            def mm2_flush(pending):
                at, g0, gsz, yps_a, yps_b = pending
                for j in range(gsz):
                    m = g0 + j
                    par = m % 2
                    dst = yps_a[0:CI + 1, :] if par == 0 else yps_b[64 : 64 + CI + 1, :]
                    nc.tensor.matmul(
                        dst,
                        gxT[:, m * (CI + 1) : (m + 1) * (CI + 1)],
                        at[:, j * NB : (j + 1) * NB],
                        start=(m == par),
                        stop=(m >= n_mb - 2),
                        tile_position=(0, 64 * par),
                    )

            def emit_partial_stats(chunks, count, cci, cco, out_tile_tag):
                """bn_aggr over bnst[:, chunks, :] -> (sum, sumsq)*count,
                DMA to DRAM, pair-allreduce, DMA result back. Returns the
                SBUF tile that will hold the pair-reduced (sum, sumsq)."""
                mv = consts.tile([C, 2], F32, tag=out_tile_tag + "_mv")
                nc.vector.bn_aggr(out=mv, in_=bnst[:, chunks, :])
                st = consts.tile([C, 2], F32, tag=out_tile_tag + "_st")
                msq = consts.tile([C, 1], F32, tag=out_tile_tag + "_msq")
                nc.vector.tensor_mul(msq, mv[:, 0:1], mv[:, 0:1])
                nc.vector.tensor_scalar_mul(st[:, 0:1], mv[:, 0:1], float(count))
                nc.vector.tensor_add(msq, msq, mv[:, 1:2])
                nc.vector.tensor_scalar_mul(st[:, 1:2], msq, float(count))
                if not with_collective:
                    return st
                # input on the sync queue; result reads on the scalar queue so
                # the second collective's input DMA never queues behind the
                # first collective's completion wait (head-of-line blocking)
                nc.sync.dma_start(out=cci[:, :], in_=st)
                emit_cc(cci, cco)
                if cc_kind == "ag_pair":
                    gath = consts.tile([C, 4], F32, tag=out_tile_tag + "_g")
                    nc.scalar.dma_start(out=gath[:, 0:2], in_=cco[0:C, :])
                    nc.scalar.dma_start(out=gath[:, 2:4], in_=cco[C : 2 * C, :])
                    red = consts.tile([C, 2], F32, tag=out_tile_tag + "_red")
                    nc.vector.tensor_add(red, gath[:, 0:2], gath[:, 2:4])
                else:
                    red = consts.tile([C, 2], F32, tag=out_tile_tag + "_red")
                    nc.scalar.dma_start(out=red, in_=cco[:, :])
                return red

            statsA = None
            for nb in range(n_nb):
                nsl = slice(nb * NB, (nb + 1) * NB)
                # two col-packed softmax-V accumulators (separate banks so the
                # two interleaved has_written groups don't clobber each other)
                yps_a = py.tile([128, NB], F32, tag="y", name="yps_a")
                yps_b = py.tile([128, NB], F32, tag="y", name="yps_b")

                # mm1 is emitted in bursts of MM1C consecutive key blocks,
                # each on its own 32-row PE band, so a burst streams through
                # the PE concurrently (~512 cycles for MM1C blocks). exp
                # groups of GRP blocks complete inside the bursts.
                MM1C = 4 if pack_mm1 else 1
                ps_tiles = {}

                def emit_group_done(g, gsz, ps):
                    nonlocal ychain, pending_wy, statsA
                    at = atp.tile([128, GRP * NB], LOWP, tag="at", bufs=3)
                    nc.scalar.activation(at[:, : gsz * NB], ps[:, : gsz * NB], AF.Exp)
                    pqueue.append((at, g * GRP, gsz, yps_a, yps_b))
                    if len(pqueue) > 1:
                        mm2_flush(pqueue.pop(0))
                    if g == 0 and ychain is not None:
                        if pending_wy is not None:
                            emit_wy_tail(*pending_wy)
                        if nb == n_nb - 1:
                            with tc.high_priority():
                                pending_wy = emit_ychain(*ychain)
                        else:
                            pending_wy = emit_ychain(*ychain)
                        ychain = None
                    elif g == 1 and nb == n_nb - 1 and pending_wy is not None:
                        # last block: pull the previous block's W-projection +
                        # stats forward so the early stats collective can
                        # launch while this block's attention still runs
                        with tc.high_priority():
                            emit_wy_tail(*pending_wy)
                        pending_wy = None
                    elif g == 2 and nb == n_nb - 1 and n_nb >= 2:
                        # chunks 0..n_nb-2 stats are complete: launch the
                        # early partial-stats collective so its latency
                        # hides under the final block's attention
                        with tc.high_priority():
                            statsA = emit_partial_stats(
                                slice(0, n_nb - 1), (n_nb - 1) * NB,
                                cca_in if with_collective else None,
                                cca_out if with_collective else None,
                                "stA",
                            )
                    elif g == 4 and pending_wy is not None:
                        # three groups after the y-chain was issued, so its
                        # serial DVE/DMA/broadcast chain is done and the PE
                        # does not stall at the W-projection matmul
                        emit_wy_tail(*pending_wy)
                        pending_wy = None

                for c0 in range(0, n_mb, MM1C):
                    if nb == 0 and deferred_chunks and \
                            c0 + 2 * MM1C > deferred_chunks[0] * mb_per_nb:
                        emit_prologue_chunk(deferred_chunks.pop(0),
                                            with_dma=False)
                    for m in range(c0, min(c0 + MM1C, n_mb)):
                        g, j = divmod(m, GRP)
                        if j == 0:
                            ps_tiles[g] = pf.tile(
                                [128, GRP * NB], F32, tag="f", name="ps_f"
                            )
                        b = m % 4
                        if pack_mm1:
                            nc.tensor.matmul(
                                ps_tiles[g][:, j * NB : (j + 1) * NB],
                                ph_rep[32 * b : 32 * (b + 1), m * MBLK : (m + 1) * MBLK],
                                th_rep[32 * b : 32 * (b + 1), nsl],
                                start=True,
                                stop=True,
                                tile_position=(32 * b, 0),
                            )
                        else:
                            nc.tensor.matmul(
                                ps_tiles[g][:, j * NB : (j + 1) * NB],
                                ph_rep[0:32, m * MBLK : (m + 1) * MBLK],
                                th_rep[0:32, nsl],
                                start=True,
                                stop=True,
                            )
                    # groups fully covered by the bursts so far
                    for g in sorted(ps_tiles):
                        gsz = min(GRP, n_mb - g * GRP)
                        if g * GRP + gsz <= min(c0 + MM1C, n_mb):
                            emit_group_done(g, gsz, ps_tiles.pop(g))
                ychain = (nb, yps_a, yps_b)
            # pin everything post-loop to a late sim release time: the Tile
            # scheduler's cost model mis-estimates DMA/PE latencies and would
            # otherwise queue these final-block ops AHEAD of the early-stats
            # chain on the DVE/sync queues, blocking the early collective
            with tc.tile_wait_until(1.0):
                while pqueue:
                    mm2_flush(pqueue.pop(0))
                if pending_wy is not None:
                    emit_wy_tail(*pending_wy)
                emit_wy_tail(*emit_ychain(*ychain))

                # ---- late partial stats (final chunk) + combine ----
                statsB = emit_partial_stats(
                    slice(n_nb - 1, n_nb), NB,
                    ccb_in if with_collective else None,
                    ccb_out if with_collective else None,
                    "stB",
                )
                allstats = consts.tile([C, 2], F32, tag="allstats")
                if statsA is not None:
                    nc.vector.tensor_add(allstats, statsA, statsB)
                else:
                    nc.vector.tensor_copy(allstats, statsB)

                # ---- BN finalize: scale = gamma*rsqrt(var+eps), shift = beta-mean*scale
                mean_t = consts.tile([C, 1], F32, tag="mean_t")
                nc.vector.tensor_scalar_mul(mean_t, allstats[:, 0:1], 1.0 / bn_count)
                var_t = consts.tile([C, 1], F32, tag="var_t")
                nc.vector.tensor_scalar_mul(var_t, allstats[:, 1:2], 1.0 / bn_count)
                msq = consts.tile([C, 1], F32, tag="msq")
                nc.vector.tensor_mul(msq, mean_t, mean_t)
                nc.vector.tensor_sub(var_t, var_t, msq)
                # rsqrt via DVE reciprocal of ACT sqrt(var+eps): the single
                # sqrt table switch is emitted right after the last main-loop
                # exp, so it executes during the ACT idle window while the
                # collective is in flight (identity for the apply phase is in
                # every table set)
                eps_t = consts.tile([C, 1], F32, tag="eps_t")
                nc.vector.memset(eps_t, BN_EPS)
                std_t = consts.tile([C, 1], F32, tag="std_t")
                nc.scalar.activation(std_t, var_t, AF.Sqrt, bias=eps_t)
                rstd = consts.tile([C, 1], F32, tag="rstd")
                nc.vector.reciprocal(rstd, std_t)
                scale_t = consts.tile([C, 1], F32, tag="scale_t")
                nc.vector.tensor_mul(scale_t, rstd, bnp[:, 0:1])
                shift_t = consts.tile([C, 1], F32, tag="shift_t")
                nc.vector.tensor_mul(shift_t, mean_t, scale_t)
                nc.vector.tensor_sub(shift_t, bnp[:, 1:2], shift_t)

                # ---- apply + residual + store ----
                APW = min(2 * NB, NT)  # apply-chunk width
                for ci, a0 in enumerate(range(0, NT, APW)):
                    nsl = slice(a0, a0 + APW)
                    o_sb = outp.tile([C, APW], F32, tag="o_sb")
                    nc.scalar.activation(
                        o_sb, wy_full[:, nsl], AF.Identity,
                        bias=shift_t, scale=scale_t,
                    )
                    nc.vector.tensor_add(o_sb, o_sb, xf[0:C, nsl])
                    nc.sync.dma_start(out=out_d[:, nsl], in_=o_sb)

    nc.compile()
    return nc


def _prep_host(x, g_w, g_b, theta_w, theta_b, phi_w, phi_b, w_w, w_b,
               bn_gamma, bn_beta):
    """Host-side weight prep + input sharding. Returns (in_maps, shapes)."""
    th_aug = np.concatenate([theta_w.T, theta_b[None, :]], axis=0)  # [65, 32]
    ph_aug = np.concatenate([phi_w.T, phi_b[None, :]], axis=0)
    threp = np.tile(th_aug, (1, 4)).astype(np.float16)  # [65, 128]
    phrep = np.tile(ph_aug, (1, 4)).astype(np.float16)
    gaug = np.zeros((C + 1, CI + 1), np.float16)
    gaug[0:C, 0:CI] = g_w.T
    gaug[C, 0:CI] = g_b
    gaug[C, CI] = 1.0
    wt = np.ascontiguousarray(w_w.T).astype(np.float16)  # [32, 64]
    bnp = np.stack([bn_gamma, bn_beta], axis=1).astype(np.float32)  # [64, 2]

    in_maps = []
    for k in range(8):
        q, b = k // 2, k % 2
        qh, qw = q // 2, q % 2
        xq = x[b, :, qh * HQ : (qh + 1) * HQ, qw * HQ : (qw + 1) * HQ]
        xq = xq.reshape(C, N_FULL).astype(np.float32)
        xq = np.concatenate([xq, np.ones((1, N_FULL), np.float32)], axis=0)
        in_maps.append(
            dict(xq=np.ascontiguousarray(xq),
                 xqlp=np.ascontiguousarray(xq.astype(np.float16)),
                 threp=threp, phrep=phrep, gaug=gaug, wt=wt, bnp=bnp)
        )
    return in_maps


_NC_CACHE = {}


def _get_nc(pack_mm1=True):
    key = ("full", pack_mm1)
    if key not in _NC_CACHE:
        _NC_CACHE[key] = build_nc(
            n_tokens=N_FULL, n_cores=8, with_collective=True, pack_mm1=pack_mm1
        )
    return _NC_CACHE[key]


def kernel_with_results(trace=False, **inputs):
    """Run on 8 cores; returns (full_output [2,64,128,128], BassKernelResults)."""
    nc = _get_nc()
    in_maps = _prep_host(**inputs)
    last_err = None
    for _attempt in range(3):
        try:
            res = run_bass_kernel_spmd(
                nc, in_maps, core_ids=list(range(8)), trace=trace
            )
            break
        except Exception as e:  # transient NRT/axon device hiccups
            last_err = e
    else:
        raise last_err
    x = inputs["x"]
    out = np.empty((B, C, H, W), np.float32)
    for k in range(8):
        q, b = k // 2, k % 2
        qh, qw = q // 2, q % 2
        blk = res.results[k]["out"].reshape(C, HQ, HQ)
        out[b, :, qh * HQ : (qh + 1) * HQ, qw * HQ : (qw + 1) * HQ] = blk
    return out.astype(x.dtype), res


def kernel(**inputs):
    out, _ = kernel_with_results(trace=False, **inputs)
    return out

